# revision 3
# baseline (speedup 1.0000x reference)
"""BERT self-attention (B=4, S=2048, HID=768, 12 heads) on 8 NeuronCores, v3.

Sharding: data-parallel over batch (4) x tensor-parallel over heads (2 groups
of 6 heads) -> 8 cores, no cross-core communication.

v3 design (Act-engine-bound, ~fp16 accuracy):
- All tensor-path data is fp16 (hsT/W/qt/kt/V/ctx), probs are bf16 (the Act
  engine's exp writes bf16 at the same 1.04us/1024 as fp32; fp16-out is
  slower). hs is transposed host-side; no on-device PE transposes at all.
- Every phase-B matmul is a [64, x] row-tile config: head h0 runs in PE array
  rows 0-63 (tile T0) and h1 in rows 64-127 (T8) CONCURRENTLY (2x scores).
  The ctx accumulation splits each 128-key chunk into two 64-key halves
  (T0 -> cpsA, T8 -> cpsB, summed by the DVE afterwards), which costs nothing
  but keeps the whole phase in one tiling mode (mode switches drain the PE).
- Projections run as row-tiled "filler units" that share the scores' PSUM
  pool and are interleaved into the Act-bound pipeline (block 0 has no ctx
  work, so the V projection and remaining Q/K slices hide there).
- Softmax tail: ctx^T (+ones-column denominator row) is summed to fp16 and
  DMA-TRANSPOSED (hwdge xbar, 16-bit) back to q-major; DVE does
  reciprocal+scale. The PE never leaves the 64-row mode.
- Software pipeline: block N = (head-pair, query-half) emits 16 score steps;
  block N's ctx matmuls run interleaved into block N+1's steps (h0 in steps
  0-7, h1 in 8-15) so the PE always has work while the Act engine paces the
  kernel (~1.04us per 1024-wide exp, 192 exps/core).
"""

import numpy as np

import concourse.bacc as bacc
import concourse.mybir as mybir
import concourse.tile as tile
from concourse.bass_utils import run_bass_kernel_spmd

F32 = mybir.dt.float32
F16 = mybir.dt.float16
BF16 = mybir.dt.bfloat16
EXP = mybir.ActivationFunctionType.Exp
ALU = mybir.AluOpType

B = 4
S = 2048
HID = 768
NH = 6              # heads per core
HD = 64
D3 = NH * HD        # 384
NCORES = 8
ST = S // 128       # 16
QB = 1024
NQB = S // QB       # 2
KC = S // 128       # 16 key chunks
HE = 80             # per-head slot in v16: v(64) | ones | 15 zeros
VW = NH * HE + 48   # 528

_nc_cache: dict = {}


def _build(ck: int):
    nc = bacc.Bacc("TRN2", target_bir_lowering=False, debug=False)
    hst_d = nc.dram_tensor("hst16", [ck * 128, S], F16, kind="ExternalInput")
    wq_d = nc.dram_tensor("wq16", [ck * 128, D3], F16, kind="ExternalInput")
    wk_d = nc.dram_tensor("wk16", [ck * 128, D3], F16, kind="ExternalInput")
    wv_d = nc.dram_tensor("wv16", [ck * 128, D3], F16, kind="ExternalInput")
    maske_d = nc.dram_tensor("maske", [128, KC], F32, kind="ExternalInput")
    out_d = nc.dram_tensor("out", [S, D3], F16, kind="ExternalOutput")

    blocks = [(hp, qb) for hp in range(NH // 2) for qb in range(NQB)]

    with tile.TileContext(nc) as tc:
        with (
            tc.tile_pool(name="const", bufs=1) as constp,
            tc.tile_pool(name="qkpool", bufs=1) as qkp,
            tc.tile_pool(name="vpool", bufs=1) as vp,
            tc.tile_pool(name="hstpool", bufs=1) as hstp,
            tc.tile_pool(name="wpool", bufs=1) as wp,
            tc.tile_pool(name="prpool", bufs=1) as prp,
            tc.tile_pool(name="outpool", bufs=1) as outp,
            tc.tile_pool(name="ctxtpool", bufs=1) as ctxtp,
            tc.tile_pool(name="tp2pool", bufs=4) as tp2p,
            tc.tile_pool(name="mergepool", bufs=4) as mgp,
            tc.tile_pool(name="rdpool", bufs=4) as rdp,
            tc.tile_pool(name="stps", bufs=3, space="PSUM") as stps,
            tc.tile_pool(name="ctxps", bufs=1, space="PSUM") as ctxps,
        ):
            maske_sb = constp.tile([128, KC], F32)
            qt = [qkp.tile([128, S], F16, name=f"qt{m}") for m in range(3)]
            kt = [qkp.tile([128, S], F16, name=f"kt{m}") for m in range(3)]
            ktpA = qkp.tile([128, S], F16, name="ktpA")
            ktpB = qkp.tile([128, S], F16, name="ktpB")
            nc.gpsimd.memset(ktpA[64:128, :], 0.0)
            nc.gpsimd.memset(ktpB[0:64, :], 0.0)
            v16 = [vp.tile([128, VW], F16, name=f"v16_{i}") for i in range(ST)]
            hst16 = hstp.tile([128, ck, S], F16, name="hst16")
            wsb = {
                'q': wp.tile([128, ck, D3], F16, name="wq16"),
                'k': wp.tile([128, ck, D3], F16, name="wk16"),
                'v': wp.tile([128, ck, D3], F16, name="wv16"),
            }
            out_sb = [outp.tile([128, D3], F16, name=f"os{i}") for i in range(ST)]

            # ---- input DMA: weights + mask first, hst16 in (n, c) pieces so
            # the first Q/K projection units can start early.
            nc.scalar.dma_start(wsb['k'][:], wk_d.ap().rearrange("(c p) n -> p c n", p=128))
            nc.scalar.dma_start(wsb['q'][:], wq_d.ap().rearrange("(c p) n -> p c n", p=128))
            nc.scalar.dma_start(maske_sb[:], maske_d[:])
            hst_r = hst_d.ap().rearrange("(c p) s -> p c s", p=128)
            QCW = 512
            for n in range(S // QCW):
                eng = nc.sync if n % 2 == 0 else nc.scalar
                for c in range(ck):
                    eng.dma_start(
                        hst16[:, c, n * QCW:(n + 1) * QCW],
                        hst_r[:, c, n * QCW:(n + 1) * QCW],
                    )
            nc.scalar.dma_start(wsb['v'][:], wv_d.ap().rearrange("(c p) n -> p c n", p=128))

            # v16 ones columns (0.5 = the Wv host-side fold) + zero padding
            for st in range(ST):
                v3 = v16[st][:, 0:NH * HE].rearrange("p (h e) -> p h e", h=NH)
                nc.gpsimd.memset(v3[:, :, HD:HD + 1], 0.5)
                nc.gpsimd.memset(v3[:, :, HD + 1:HE], 0.0)
                nc.gpsimd.memset(v16[st][:, NH * HE:VW], 0.0)

            # ---------------- unit emitters (all 64-row tile configs) -------
            def emit_qk_unit(m, n, qk):
                """Q/K projection for output rows m*128..m*128+127, seq cols
                n*512..n*512+511. T0/T8 halves land in the two banks of one
                sps tile; DVE adds them into qt/kt fp16."""
                ps = stps.tile([128, QB], F32, name="sps")
                dst = qt[m] if qk == 'q' else kt[m]
                w16 = wsb[qk]
                for c in range(ck):
                    nc.tensor.matmul(
                        ps[:, 0:512],
                        w16[:, c, m * 128:(m + 1) * 128],
                        hst16[:, c, n * 512:(n + 1) * 512],
                        start=(c == 0), stop=(c == ck - 1),
                    )
                nc.vector.tensor_copy(dst[:, n * 512:(n + 1) * 512], ps[:, 0:512])

            def emit_v_unit(st):
                """V projection for seq tile st -> v16[st] head slots."""
                ps = stps.tile([128, QB], F32, name="sps")
                for c in range(ck):
                    nc.tensor.matmul(
                        ps[:, 0:D3],
                        hst16[:, c, st * 128:(st + 1) * 128],
                        wsb['v'][:, c, :],
                        start=(c == 0), stop=(c == ck - 1),
                    )
                v3 = v16[st][:, 0:NH * HE].rearrange("p (h e) -> p h e", h=NH)
                nc.vector.tensor_copy(
                    v3[:, :, 0:HD],
                    ps[:, 0:D3].rearrange("p (h d) -> p h d", h=NH),
                )

            def emit_scores_step(hp, qb, kc, prs):
                """Both heads' scores for key chunk kc + exp -> bf16 probs."""
                sp2 = [stps.tile([128, QB], F32, name="sps") for _ in range(2)]
                for qc in range(2):
                    for hh in range(2):
                        ktp = ktpA if hh == 0 else ktpB
                        nc.tensor.matmul(
                            sp2[hh][:, qc * 512:(qc + 1) * 512],
                            ktp[:, kc * 128:(kc + 1) * 128],
                            qt[hp][:, qb * QB + qc * 512:qb * QB + (qc + 1) * 512],
                            start=True, stop=True,
                        )
                for hh in range(2):
                    nc.scalar.activation(
                        prs[hh][:, kc, :], sp2[hh][:], EXP,
                        bias=maske_sb[:, kc:kc + 1], scale=1.0,
                    )

            st_heads_left = [NH] * ST

            def emit_tails(h, qb, ctxt):
                for qs in range(QB // 128):
                    sti = qb * (QB // 128) + qs
                    tp2 = tp2p.tile([128, HE], F16, name="tp2")
                    nc.sync.dma_start_transpose(
                        tp2[:], ctxt[:, qs * 128:(qs + 1) * 128])
                    rd = rdp.tile([128, 1], F32, name="rd")
                    nc.vector.reciprocal(rd[:], tp2[:, HD:HD + 1])
                    nc.vector.tensor_scalar_mul(
                        out_sb[sti][:, h * HD:(h + 1) * HD],
                        tp2[:, 0:HD], rd[:],
                    )
                    st_heads_left[sti] -= 1
                    if st_heads_left[sti] == 0:
                        nc.sync.dma_start(
                            out_d[sti * 128:(sti + 1) * 128, :], out_sb[sti][:])

            cps_live = {}

            def emit_ctx_unit(blk, hh, kc):
                """ctx for head (blk.hp*2+hh), key chunk kc: T0 takes keys
                0-63 -> cpsA, T8 keys 64-127 -> cpsB; after kc==15, DVE adds
                the halves into fp16 ctxt and the tails run."""
                hp, qb, prs = blk
                h = hp * 2 + hh
                if kc == 0:
                    cps_live['A'] = ctxps.tile([128, QB], F32, name="cpsA")
                cps = cps_live['A']
                pr = prs[hh]
                for qc in range(2):
                    nc.tensor.matmul(
                        cps[:, qc * 512:(qc + 1) * 512],
                        v16[kc][:, h * HE:h * HE + 128],
                        pr[:, kc, qc * 512:(qc + 1) * 512],
                        start=(kc == 0), stop=(kc == KC - 1),
                    )
                if kc == KC - 1:
                    ctxt = ctxtp.tile([HE, QB], F16, name="ctxt", bufs=2)
                    nc.vector.tensor_copy(ctxt[:], cps[0:HE, :])
                    emit_tails(h, qb, ctxt)

            # ---------------- schedule ----------------
            # pre-units: enough Q/K projection for block (0, 0)'s scores
            pre = [('qk', 0, n, 'k') for n in range(4)] + \
                  [('qk', 0, 0, 'q'), ('qk', 0, 1, 'q')]
            filler = [('qk', 0, 2, 'q'), ('qk', 0, 3, 'q')]
            filler += [('v', st) for st in range(ST)]
            for m in (1, 2):
                filler += [('qk', m, n, k) for n in range(4) for k in ('k', 'q')]
            filler = list(reversed(filler))  # pop() from the front-priority end

            def emit_unit(u):
                if u[0] == 'qk':
                    emit_qk_unit(u[1], u[2], u[3])
                else:
                    emit_v_unit(u[1])

            for u in pre:
                emit_unit(u)

            nc.vector.tensor_copy(ktpA[0:64, :], kt[0][0:64, :])
            nc.vector.tensor_copy(ktpB[64:128, :], kt[0][64:128, :])
            from collections import deque
            prev = None
            for bi, (hp, qb) in enumerate(blocks):
                prs = [
                    prp.tile([128, KC, QB], BF16, name="pr_h0", bufs=1),
                    prp.tile([128, KC, QB], BF16, name="pr_h1", bufs=2),
                ]
                cur = (hp, qb, prs)
                last = bi == len(blocks) - 1
                pq = deque((hh, kc) for hh in range(2) for kc in range(KC)) \
                    if prev is not None else deque()
                selfq = deque(range(KC)) if last else deque()
                for kci in range(KC):
                    emit_scores_step(hp, qb, kci, prs)
                    if kci % 2 == 1:
                        if prev is not None:
                            n = 4 if not last else (6 if kci <= 7 else 4)
                            for _ in range(n):
                                if pq:
                                    hh, kc = pq.popleft()
                                    emit_ctx_unit(prev, hh, kc)
                            if last and not pq:
                                while selfq and selfq[0] <= kci - 3 and \
                                        len(selfq) > KC - 1 - (kci - 3):
                                    emit_ctx_unit(cur, 0, selfq.popleft())
                            if filler and kci % 4 == 3:
                                emit_unit(filler.pop())
                        else:
                            for _ in range(4):
                                if filler:
                                    emit_unit(filler.pop())
                if qb == NQB - 1 and hp + 1 < NH // 2:
                    nc.vector.tensor_copy(ktpA[0:64, :], kt[hp + 1][0:64, :])
                    nc.vector.tensor_copy(ktpB[64:128, :], kt[hp + 1][64:128, :])
                prev = cur

            for kc in selfq:
                emit_ctx_unit(prev, 0, kc)
            for kc in range(KC):
                emit_ctx_unit(prev, 1, kc)

    nc.compile()
    return nc


def _get_nc(ck: int):
    if ck not in _nc_cache:
        _nc_cache[ck] = _build(ck)
    return _nc_cache[ck]


def _prepare_in_maps(hidden_states, attention_mask, Wq, bq, Wk, bk, Wv, bv):
    hs = np.asarray(hidden_states, dtype=np.float32)
    mask = np.asarray(attention_mask, dtype=np.float32).reshape(B, S)
    wq = np.asarray(Wq, dtype=np.float32) * np.float32(0.125)
    wk = np.asarray(Wk, dtype=np.float32)
    wv = np.asarray(Wv, dtype=np.float32) * np.float32(0.5)
    bqs = np.asarray(bq, dtype=np.float32) * np.float32(0.125)
    bks = np.asarray(bk, dtype=np.float32)
    bvs = np.asarray(bv, dtype=np.float32) * np.float32(0.5)

    if bqs.any() or bks.any() or bvs.any():
        ck = 7
        pad = ck * 128 - (HID + 1)
        hs = np.concatenate(
            [hs, np.ones((B, S, 1), np.float32),
             np.zeros((B, S, pad), np.float32)], axis=2)

        def aug(w, b):
            return np.concatenate(
                [w, b[None, :], np.zeros((pad, HID), np.float32)], axis=0)
        wq, wk, wv = aug(wq, bqs), aug(wk, bks), aug(wv, bvs)
    else:
        ck = 6

    hst16 = np.ascontiguousarray(hs.transpose(0, 2, 1)).astype(np.float16)
    wq16 = wq.astype(np.float16)
    wk16 = wk.astype(np.float16)
    wv16 = wv.astype(np.float16)

    in_maps = []
    for core in range(NCORES):
        b, hg = core // 2, core % 2
        cols = slice(hg * D3, (hg + 1) * D3)
        in_maps.append({
            "hst16": np.ascontiguousarray(hst16[b]),
            "wq16": np.ascontiguousarray(wq16[:, cols]),
            "wk16": np.ascontiguousarray(wk16[:, cols]),
            "wv16": np.ascontiguousarray(wv16[:, cols]),
            "maske": np.ascontiguousarray(mask[b].reshape(KC, 128).T),
        })
    return ck, in_maps


def run(hidden_states, attention_mask, Wq, bq, Wk, bk, Wv, bv, **rb_kwargs):
    ck, in_maps = _prepare_in_maps(
        hidden_states, attention_mask, Wq, bq, Wk, bk, Wv, bv)
    nc = _get_nc(ck)
    res = run_bass_kernel_spmd(nc, in_maps, core_ids=list(range(NCORES)), **rb_kwargs)
    out = np.empty((B, S, HID), dtype=np.float32)
    for core in range(NCORES):
        b, hg = core // 2, core % 2
        out[b, :, hg * D3:(hg + 1) * D3] = res.results[core]["out"].astype(np.float32)
    return out, res


def kernel(hidden_states, attention_mask, Wq, bq, Wk, bk, Wv, bv):
    out, _ = run(hidden_states, attention_mask, Wq, bq, Wk, bk, Wv, bv)
    return out
